# revision 35
# baseline (speedup 1.0000x reference)
"""Trainium2 Bass kernel: MoE transition/reward model, data-parallel on 8 cores.

Layout: all on-device compute is feature-major (features on SBUF partitions,
batch rows on the free dim); the host transposes inputs/outputs, which keeps
every matmul in the natural lhsT=weight / rhs=activation form with zero
on-device transposes. Matmuls run in float32r (full-rate fp32 mode).
"""
import numpy as np

N = 16384
LATENT = 512
ACTION = 64
HIDDEN = 1024
E = 8
RDIM = 101
N_CORES = 8
ROWS = N // N_CORES            # rows per core
PASS_ROWS = 1024               # rows per weight-streaming pass
CHUNK = 512                    # matmul moving free dim
KD = LATENT // 128             # 4 z k-chunks
KH = HIDDEN // 128             # 8 hidden k-chunks

_cache = {}


def _emit(nc, tc, io, rows, ctx):
    import concourse.mybir as mybir
    from concourse.mybir import AluOpType as Op
    from concourse.mybir import ActivationFunctionType as Act
    f32 = mybir.dt.float32
    f32r = mybir.dt.float32r

    n_pass = rows // PASS_ROWS if rows >= PASS_ROWS else 1
    pass_rows = min(rows, PASS_ROWS)
    n_chunk = pass_rows // CHUNK

    Pc = ctx.enter_context(tc.tile_pool(name="consts", bufs=1))
    P1 = ctx.enter_context(tc.tile_pool(name="p1", bufs=1))
    Pws = ctx.enter_context(tc.tile_pool(name="ws", bufs=5))
    Ptmp = ctx.enter_context(tc.tile_pool(name="tmp", bufs=4))
    Pbc = ctx.enter_context(tc.tile_pool(name="bc", bufs=4))
    Pout = ctx.enter_context(tc.tile_pool(name="out", bufs=2))
    Pps = ctx.enter_context(tc.tile_pool(name="ps", bufs=8, space="PSUM"))

    # ---- constants / small weights (loaded once) ----
    g2w = Pc.tile([128, KH * 16], f32r, name="g2w")
    nc.gpsimd.dma_start(g2w[:], io["g2_W"].rearrange("(k p) m -> p k m", p=128))
    ones8 = Pc.tile([8, 1], f32r, name="ones8")
    nc.gpsimd.dma_start(ones8[:], io["c_ones8"][:])
    ones18 = Pc.tile([1, 8], f32r, name="ones18")
    nc.gpsimd.dma_start(ones18[:], io["c_ones18"][:])
    ones1x128 = Pc.tile([1, 128], f32r, name="ones1x128")
    nc.gpsimd.dma_start(ones1x128[:], io["c_ones1x128"][:])
    sel = Pc.tile([8, E * 128], f32r, name="sel")
    nc.gpsimd.dma_start(sel[:], io["c_sel"][:])
    ebt = Pc.tile([8, LATENT], f32r, name="ebt")
    nc.gpsimd.dma_start(ebt[:], io["eo_b"][:])
    preb = Pc.tile([128, 8], f32, name="preb")
    nc.gpsimd.dma_start(preb[:], io["pre_b"].rearrange("o p -> p o"))
    g1b = Pc.tile([128, 8], f32, name="g1b")
    nc.gpsimd.dma_start(g1b[:], io["g1_b"].rearrange("o p -> p o"))
    g2bd = Pc.tile([8, 1], f32, name="g2bd")
    nc.gpsimd.dma_start(g2bd[:], io["g2_b"][0:8, :])
    g2br = Pc.tile([8, 1], f32, name="g2br")
    nc.gpsimd.dma_start(g2br[:], io["g2_b"][8:16, :])
    hb0 = Pc.tile([128, 4], f32, name="hb0")
    nc.gpsimd.dma_start(hb0[:], io["head_b0"].rearrange("o p -> p o"))
    hb1 = Pc.tile([RDIM, 1], f32, name="hb1")
    nc.gpsimd.dma_start(hb1[:], io["head_b1"].rearrange("o p -> p o"))

    e1W = io["e1_W"]   # [E, H, H] f32r
    e2W = io["e2_W"]
    eoW = io["eo_W"]   # [E, H, LATENT]

    for p in range(n_pass):
        ps = p * pass_rows

        # ================= phase A: inputs, pre_proj, gate =================
        # zt alternates tags by pass parity so the next pass's z can prefetch
        # while this pass's accumulators still occupy "ztacc".
        zt = P1.tile([128, KD * pass_rows], f32r,
                     tag=("ztacc" if p % 2 == 0 else "m1b"), name=f"zt{p}")
        for k in range(KD):
            nc.sync.dma_start(zt[:, k * pass_rows:(k + 1) * pass_rows],
                              io["zT"][k * 128:(k + 1) * 128, ps:ps + pass_rows])
        at = P1.tile([64, pass_rows], f32r, tag="at", name=f"at{p}")
        nc.sync.dma_start(at[:], io["aT"][:, ps:ps + pass_rows])
        pw = P1.tile([128, 5 * 1024], f32r, tag="A1", name=f"pw{p}")
        for k in range(5):
            kp = 128 if k < 4 else 64
            nc.sync.dma_start(pw[:kp, k * 1024:(k + 1) * 1024],
                              io["pre_W"][k * 128:k * 128 + kp, :])
        g1w = P1.tile([128, KH * 1024], f32r, tag="A2", name=f"g1w{p}")
        ht = P1.tile([128, KH * pass_rows], f32r, tag="ht", name=f"ht{p}")

        # pre_proj: h = [z;a] @ pre_W + pre_b   (no relu)
        for og in range(4):
            pss = {}
            for o2 in range(2):
                for r in range(n_chunk):
                    pss[o2, r] = Pps.tile([128, CHUNK], f32, tag="ps",
                                          name=f"pp{p}_{og}_{o2}_{r}")
            for k in range(5):
                kp = 128 if k < 4 else 64
                for o2 in range(2):
                    o = og * 2 + o2
                    for r in range(n_chunk):
                        if k < 4:
                            rhs = zt[:, k * pass_rows + r * CHUNK:
                                     k * pass_rows + r * CHUNK + CHUNK]
                        else:
                            rhs = at[:, r * CHUNK:r * CHUNK + CHUNK]
                        nc.tensor.matmul(pss[o2, r][:],
                                         pw[:kp, k * 1024 + o * 128:k * 1024 + o * 128 + 128],
                                         rhs, start=(k == 0), stop=(k == 4))
            for o2 in range(2):
                o = og * 2 + o2
                for r in range(n_chunk):
                    nc.vector.tensor_scalar_add(
                        ht[:, o * pass_rows + r * CHUNK:o * pass_rows + r * CHUNK + CHUNK],
                        pss[o2, r][:], preb[:, o:o + 1])
            if og == 0:
                # emitted after the first pre og so the SP DMA ring serves
                # zt/at/pw (needed first) before this 4MB transfer
                nc.sync.dma_start(g1w[:],
                                  io["g1_W"].rearrange("(k p) m -> p k m", p=128))

        # gate MLP: g1 with both row-chunks inside each og group (longer PE
        # bursts; copybacks hide). m1 per row-chunk on separate tags.
        wd = P1.tile([8, pass_rows], f32r, tag="at", name=f"wd{p}")
        wr = P1.tile([8, pass_rows], f32r, tag="wr", name=f"wr{p}")
        m1s = []
        for r in range(n_chunk):
            m1s.append(P1.tile([128, KH * CHUNK], f32r,
                               tag=("accr" if r == 0 else "m1b"),
                               name=f"m1_{p}_{r}"))
        for og in range(4):
            pss = {}
            for o2 in range(2):
                for r in range(n_chunk):
                    pss[o2, r] = Pps.tile([128, CHUNK], f32, tag="ps",
                                          name=f"pg{p}_{og}_{o2}_{r}")
            for k in range(KH):
                for o2 in range(2):
                    o = og * 2 + o2
                    for r in range(n_chunk):
                        nc.tensor.matmul(
                            pss[o2, r][:],
                            g1w[:, k * 1024 + o * 128:k * 1024 + o * 128 + 128],
                            ht[:, k * pass_rows + r * CHUNK:k * pass_rows + r * CHUNK + CHUNK],
                            start=(k == 0), stop=(k == KH - 1))
            for o2 in range(2):
                o = og * 2 + o2
                for r in range(n_chunk):
                    nc.vector.tensor_scalar(
                        m1s[r][:, o * CHUNK:(o + 1) * CHUNK],
                        pss[o2, r][:], g1b[:, o:o + 1], 0.0,
                        op0=Op.add, op1=Op.max)
        # Softmax with deferred normalization: wd/wr hold UNNORMALIZED exp;
        # 1/sum lands in recd/recr and is applied to the head outputs in
        # phase C (a per-column scale commutes through the head matmul).
        # This keeps the slow 1-partition reciprocal off the PE critical path.
        rec_all = P1.tile([1, 2 * pass_rows], f32r, tag="rec", name=f"rec{p}")
        recd = rec_all[:, 0:pass_rows]
        recr = rec_all[:, pass_rows:2 * pass_rows]
        gcfg = [(r, gi) for r in range(n_chunk) for gi in range(2)]
        gmeta = {0: ((0, 8), g2bd, wd, recd, io["wdynT"]),
                 1: ((8, 16), g2br, wr, recr, io["wrewT"])}
        pgs = {}
        for r, gi in gcfg:
            gslice = gmeta[gi][0]
            pg = Pps.tile([8, CHUNK], f32, tag="ps", name=f"pgg{p}_{r}_{gi}")
            for k in range(KH):
                nc.tensor.matmul(
                    pg[:], g2w[:, k * 16 + gslice[0]:k * 16 + gslice[1]],
                    m1s[r][:, k * CHUNK:(k + 1) * CHUNK],
                    start=(k == 0), stop=(k == KH - 1))
            pgs[r, gi] = pg
            nc.scalar.activation(gmeta[gi][2][:, r * CHUNK:(r + 1) * CHUNK],
                                 pg[:], Act.Exp, bias=gmeta[gi][1][:, 0:1])
        scps = {}
        for r, gi in gcfg:
            wt = gmeta[gi][2]
            psums = Pps.tile([1, CHUNK], f32, tag="ps", name=f"psum{p}_{r}_{gi}")
            nc.tensor.matmul(psums[:], ones8[:],
                             wt[:, r * CHUNK:(r + 1) * CHUNK],
                             start=True, stop=True)
            # fast PSUM->SBUF copy on ACT releases the bank; the slow
            # 1-partition reciprocal then runs off the critical path
            scp = Ptmp.tile([1, CHUNK], f32, tag="tmp", name=f"scp{p}_{r}_{gi}")
            nc.scalar.activation(scp[:], psums[:], Act.Copy)
            scps[r, gi] = scp

        # ================= phase B: experts =================
        accd = P1.tile([128, KD * pass_rows], f32r, tag="ztacc", name=f"accd{p}")
        accr = P1.tile([128, KD * pass_rows], f32r, tag="accr", name=f"accr{p}")
        # init acc with gate-weighted eo bias: acc[d, n] = sum_e eo_b[e, d] w[e, n]
        for r in range(n_chunk):
            for d in range(KD):
                pi = Pps.tile([128, CHUNK], f32, tag="ps", name=f"pid{p}_{r}_{d}")
                nc.tensor.matmul(pi[:], ebt[:, d * 128:(d + 1) * 128],
                                 wd[:, r * CHUNK:(r + 1) * CHUNK],
                                 start=True, stop=True)
                nc.scalar.activation(
                    accd[:, d * pass_rows + r * CHUNK:d * pass_rows + r * CHUNK + CHUNK],
                    pi[:], Act.Copy)
                pi2 = Pps.tile([128, CHUNK], f32, tag="ps", name=f"pir{p}_{r}_{d}")
                nc.tensor.matmul(pi2[:], ebt[:, d * 128:(d + 1) * 128],
                                 wr[:, r * CHUNK:(r + 1) * CHUNK],
                                 start=True, stop=True)
                nc.scalar.activation(
                    accr[:, d * pass_rows + r * CHUNK:d * pass_rows + r * CHUNK + CHUNK],
                    pi2[:], Act.Copy)

        # reciprocals last on DVE: slow 1-partition ops, consumers are far away
        for r, gi in gcfg:
            rec = gmeta[gi][3]
            with nc.allow_low_precision(reason="f32r round of softmax recip"):
                nc.vector.reciprocal(rec[:, r * CHUNK:(r + 1) * CHUNK],
                                     scps[r, gi][:])

        for e in range(E):
            # broadcast gate weights across partitions via one-hot matmul
            bc = {}
            for gi, wt in ((0, wd), (1, wr)):
                for r in range(n_chunk):
                    pb = Pps.tile([128, CHUNK], f32, tag="ps",
                                  name=f"pb{p}_{e}_{gi}_{r}")
                    nc.tensor.matmul(pb[:], sel[:, e * 128:(e + 1) * 128],
                                     wt[:, r * CHUNK:(r + 1) * CHUNK],
                                     start=True, stop=True)
                    t = Pbc.tile([128, CHUNK], f32, tag="bc",
                                 name=f"bc{p}_{e}_{gi}_{r}")
                    nc.vector.tensor_copy(t[:], pb[:])
                    bc[gi, r] = t
            e1b = Pws.tile([128, 8], f32, tag="eb1", name=f"e1b{p}_{e}")
            nc.gpsimd.dma_start(e1b[:], io["e1_b"][e].rearrange("(o p) -> p o", p=128))
            e2b = Pws.tile([128, 8], f32, tag="eb2", name=f"e2b{p}_{e}")
            nc.gpsimd.dma_start(e2b[:], io["e2_b"][e].rearrange("(o p) -> p o", p=128))

            # x1 = relu(e1_W[e].T @ h + e1_b[e])
            x1 = P1.tile([128, KH * pass_rows], f32r, tag="A1", name=f"x1_{p}_{e}")
            for og in range(4):
                wsth = []
                for hh in range(2):
                    t = Pws.tile([128, 4 * 256], f32r, tag="ws",
                                 name=f"w1_{p}_{e}_{og}_{hh}")
                    nc.sync.dma_start(
                        t[:], e1W[e, hh * 512:(hh + 1) * 512,
                                  og * 256:(og + 1) * 256]
                        .rearrange("(k p) m -> p k m", p=128))
                    wsth.append(t)
                pss = {}
                for o2 in range(2):
                    for r in range(n_chunk):
                        pss[o2, r] = Pps.tile([128, CHUNK], f32, tag="ps",
                                              name=f"px1{p}_{e}_{og}_{o2}_{r}")
                for k in range(KH):
                    for o2 in range(2):
                        for r in range(n_chunk):
                            nc.tensor.matmul(
                                pss[o2, r][:],
                                wsth[k // 4][:, (k % 4) * 256 + o2 * 128:(k % 4) * 256 + o2 * 128 + 128],
                                ht[:, k * pass_rows + r * CHUNK:k * pass_rows + r * CHUNK + CHUNK],
                                start=(k == 0), stop=(k == KH - 1))
                for o2 in range(2):
                    o = og * 2 + o2
                    for r in range(n_chunk):
                        nc.scalar.activation(
                            x1[:, o * pass_rows + r * CHUNK:o * pass_rows + r * CHUNK + CHUNK],
                            pss[o2, r][:], Act.Relu, bias=e1b[:, o:o + 1])

            if e == 0:
                # normalized gate-weight outputs; emitted an expert-stage in
                # so the reciprocals (slow, DVE) are long done
                for r, gi in gcfg:
                    wt, rec, wout = gmeta[gi][2], gmeta[gi][3], gmeta[gi][4]
                    pb8 = Pps.tile([8, CHUNK], f32, tag="ps",
                                   name=f"pb8{p}_{r}_{gi}")
                    nc.tensor.matmul(pb8[:], ones18[:],
                                     rec[:, r * CHUNK:(r + 1) * CHUNK],
                                     start=True, stop=True)
                    wn = Pout.tile([8, CHUNK], f32r, tag="out",
                                   name=f"wn{p}_{r}_{gi}")
                    nc.vector.tensor_tensor(wn[:],
                                            wt[:, r * CHUNK:(r + 1) * CHUNK],
                                            pb8[:], op=Op.mult)
                    nc.scalar.dma_start(
                        wout[:, ps + r * CHUNK:ps + (r + 1) * CHUNK], wn[:])

            # x2 = relu(e2_W[e].T @ x1 + e2_b[e])
            x2 = P1.tile([128, KH * pass_rows], f32r, tag="A2", name=f"x2_{p}_{e}")
            for og in range(4):
                wsth = []
                for hh in range(2):
                    t = Pws.tile([128, 4 * 256], f32r, tag="ws",
                                 name=f"w2_{p}_{e}_{og}_{hh}")
                    nc.sync.dma_start(
                        t[:], e2W[e, hh * 512:(hh + 1) * 512,
                                  og * 256:(og + 1) * 256]
                        .rearrange("(k p) m -> p k m", p=128))
                    wsth.append(t)
                pss = {}
                for o2 in range(2):
                    for r in range(n_chunk):
                        pss[o2, r] = Pps.tile([128, CHUNK], f32, tag="ps",
                                              name=f"px2{p}_{e}_{og}_{o2}_{r}")
                for k in range(KH):
                    for o2 in range(2):
                        for r in range(n_chunk):
                            nc.tensor.matmul(
                                pss[o2, r][:],
                                wsth[k // 4][:, (k % 4) * 256 + o2 * 128:(k % 4) * 256 + o2 * 128 + 128],
                                x1[:, k * pass_rows + r * CHUNK:k * pass_rows + r * CHUNK + CHUNK],
                                start=(k == 0), stop=(k == KH - 1))
                for o2 in range(2):
                    o = og * 2 + o2
                    for r in range(n_chunk):
                        nc.vector.tensor_scalar(
                            x2[:, o * pass_rows + r * CHUNK:o * pass_rows + r * CHUNK + CHUNK],
                            pss[o2, r][:], e2b[:, o:o + 1], 0.0,
                            op0=Op.add, op1=Op.max)

            # feats = eo_W[e].T @ x2; acc += w[e] * feats (both gates)
            for dg in range(2):
                wsth = []
                for hh in range(2):
                    t = Pws.tile([128, 4 * 256], f32r, tag="ws",
                                 name=f"wo_{p}_{e}_{dg}_{hh}")
                    nc.sync.dma_start(
                        t[:], eoW[e, hh * 512:(hh + 1) * 512,
                                  dg * 256:(dg + 1) * 256]
                        .rearrange("(k p) m -> p k m", p=128))
                    wsth.append(t)
                pfs = {}
                for d2 in range(2):
                    for r in range(n_chunk):
                        pfs[d2, r] = Pps.tile([128, CHUNK], f32, tag="ps",
                                              name=f"pf{p}_{e}_{dg}_{d2}_{r}")
                for k in range(KH):
                    for d2 in range(2):
                        for r in range(n_chunk):
                            nc.tensor.matmul(
                                pfs[d2, r][:],
                                wsth[k // 4][:, (k % 4) * 256 + d2 * 128:(k % 4) * 256 + d2 * 128 + 128],
                                x2[:, k * pass_rows + r * CHUNK:k * pass_rows + r * CHUNK + CHUNK],
                                start=(k == 0), stop=(k == KH - 1))
                for d2 in range(2):
                    d = dg * 2 + d2
                    for r in range(n_chunk):
                        c0 = d * pass_rows + r * CHUNK
                        tmp = Ptmp.tile([128, CHUNK], f32, tag="tmp",
                                        name=f"td{p}_{e}_{dg}_{d2}_{r}")
                        nc.vector.tensor_tensor(tmp[:], pfs[d2, r][:],
                                                bc[0, r][:], op=Op.mult)
                        # acc += tmp on GpSimd: keeps DVE free for the PSUM
                        # reads, so PSUM banks recycle fast at eo boundaries
                        nc.gpsimd.tensor_add(accd[:, c0:c0 + CHUNK],
                                             accd[:, c0:c0 + CHUNK], tmp[:])
                        tmp2 = Ptmp.tile([128, CHUNK], f32, tag="tmp",
                                         name=f"tr{p}_{e}_{dg}_{d2}_{r}")
                        nc.vector.tensor_tensor(tmp2[:], pfs[d2, r][:],
                                                bc[1, r][:], op=Op.mult)
                        nc.gpsimd.tensor_add(accr[:, c0:c0 + CHUNK],
                                             accr[:, c0:c0 + CHUNK], tmp2[:])

        # ================= phase C: fused heads =================
        hw0h = []
        for hh in range(2):
            t = Pws.tile([128, 2 * 512], f32r, tag="ws", name=f"hw0_{p}_{hh}")
            nc.sync.dma_start(
                t[:], io["head_W0"][hh * 256:(hh + 1) * 256, :]
                .rearrange("(k p) m -> p k m", p=128))
            hw0h.append(t)
        hw1 = Pws.tile([128, 4 * RDIM], f32r, tag="ws", name=f"hw1_{p}")
        nc.sync.dma_start(hw1[:], io["head_W1"].rearrange("(k p) m -> p k m", p=128))
        for r in range(n_chunk):
            # broadcast 1/sum to 128 partitions for the deferred normalization
            rb = {}
            for gi, rec in ((0, recd), (1, recr)):
                pbc = Pps.tile([128, CHUNK], f32, tag="ps", name=f"pbc{p}_{r}_{gi}")
                nc.tensor.matmul(pbc[:], ones1x128[:],
                                 rec[:, r * CHUNK:(r + 1) * CHUNK],
                                 start=True, stop=True)
                t = Pbc.tile([128, CHUNK], f32, tag="bc", name=f"rb{p}_{r}_{gi}")
                nc.vector.tensor_copy(t[:], pbc[:])
                rb[gi] = t
            for d in range(4):
                ph = Pps.tile([128, CHUNK], f32, tag="ps", name=f"ph{p}_{r}_{d}")
                for k in range(KD):
                    nc.tensor.matmul(
                        ph[:], hw0h[k // 2][:, (k % 2) * 512 + d * 128:(k % 2) * 512 + d * 128 + 128],
                        accd[:, k * pass_rows + r * CHUNK:k * pass_rows + r * CHUNK + CHUNK],
                        start=(k == 0), stop=(k == KD - 1))
                t1 = Ptmp.tile([128, CHUNK], f32, tag="tmp", name=f"th{p}_{r}_{d}")
                nc.vector.tensor_tensor(t1[:], ph[:], rb[0][:], op=Op.mult)
                ot = Pout.tile([128, CHUNK], f32, tag="out", name=f"ot{p}_{r}_{d}")
                nc.vector.tensor_scalar_add(ot[:], t1[:], hb0[:, d:d + 1])
                nc.scalar.dma_start(
                    io["nzT"][d * 128:(d + 1) * 128, ps + r * CHUNK:ps + r * CHUNK + CHUNK],
                    ot[:])
            pr = Pps.tile([RDIM, CHUNK], f32, tag="ps", name=f"prew{p}_{r}")
            for k in range(KD):
                nc.tensor.matmul(
                    pr[:], hw1[:, k * RDIM:(k + 1) * RDIM],
                    accr[:, k * pass_rows + r * CHUNK:k * pass_rows + r * CHUNK + CHUNK],
                    start=(k == 0), stop=(k == KD - 1))
            t2 = Ptmp.tile([RDIM, CHUNK], f32, tag="tmp", name=f"thr{p}_{r}")
            nc.vector.tensor_tensor(t2[:], pr[:], rb[1][0:RDIM, :], op=Op.mult)
            orw = Pout.tile([RDIM, CHUNK], f32, tag="out", name=f"orw{p}_{r}")
            nc.vector.tensor_scalar_add(orw[:], t2[:], hb1[:, 0:1])
            nc.scalar.dma_start(
                io["rewT"][:, ps + r * CHUNK:ps + r * CHUNK + CHUNK], orw[:])


def _build(rows, n_cores):
    import concourse.bacc as bacc
    import concourse.tile as tile
    import concourse.mybir as mybir
    f32 = mybir.dt.float32
    f32r = mybir.dt.float32r

    nc = bacc.Bacc("TRN2", target_bir_lowering=False, debug=False,
                   num_devices=n_cores)
    io = {}

    def inp(name, shape, dt):
        io[name] = nc.dram_tensor(name, shape, dt, kind="ExternalInput").ap()

    def outp(name, shape, dt):
        io[name] = nc.dram_tensor(name, shape, dt, kind="ExternalOutput").ap()

    inp("zT", [LATENT, rows], f32r)
    inp("aT", [ACTION, rows], f32r)
    inp("pre_W", [LATENT + ACTION, HIDDEN], f32r)
    inp("pre_b", [8, 128], f32)
    inp("g1_W", [HIDDEN, HIDDEN], f32r)
    inp("g1_b", [8, 128], f32)
    inp("g2_W", [HIDDEN, 2 * E], f32r)
    inp("g2_b", [2 * E, 1], f32)
    inp("e1_W", [E, HIDDEN, HIDDEN], f32r)
    inp("e1_b", [E, HIDDEN], f32)
    inp("e2_W", [E, HIDDEN, HIDDEN], f32r)
    inp("e2_b", [E, HIDDEN], f32)
    inp("eo_W", [E, HIDDEN, LATENT], f32r)
    inp("eo_b", [E, LATENT], f32r)
    inp("head_W0", [LATENT, LATENT], f32r)
    inp("head_W1", [LATENT, RDIM], f32r)
    inp("head_b0", [4, 128], f32)
    inp("head_b1", [RDIM, 1], f32)
    inp("c_ones8", [8, 1], f32r)
    inp("c_ones18", [1, 8], f32r)
    inp("c_ones1x128", [1, 128], f32r)
    inp("c_sel", [8, E * 128], f32r)

    outp("nzT", [LATENT, rows], f32)
    outp("rewT", [RDIM, rows], f32)
    outp("wdynT", [E, rows], f32r)
    outp("wrewT", [E, rows], f32r)

    from contextlib import ExitStack
    with tile.TileContext(nc) as tc:
        with ExitStack() as ctx:
            _emit(nc, tc, io, rows, ctx)
    nc.compile()
    return nc


def _host_inputs(z, a, pre_W, pre_b, g1_W, g1_b, g2_W, g2_b,
                 e1_W, e1_b, e2_W, e2_b, eo_W, eo_b, head_W, head_b,
                 rows, n_cores):
    """Build per-core in_maps (host-side transposes/shaping)."""
    zT = np.ascontiguousarray(z.T)
    aT = np.ascontiguousarray(a.T)
    sel_np = np.zeros((8, E * 128), np.float32)
    for e in range(E):
        sel_np[e, e * 128:(e + 1) * 128] = 1.0
    shared = {
        "pre_W": np.ascontiguousarray(pre_W),
        "pre_b": np.ascontiguousarray(pre_b.reshape(8, 128)),
        "g1_W": np.ascontiguousarray(g1_W),
        "g1_b": np.ascontiguousarray(g1_b.reshape(8, 128)),
        "g2_W": np.ascontiguousarray(g2_W),
        "g2_b": np.ascontiguousarray(g2_b.reshape(2 * E, 1)),
        "e1_W": np.ascontiguousarray(e1_W),
        "e1_b": np.ascontiguousarray(e1_b),
        "e2_W": np.ascontiguousarray(e2_W),
        "e2_b": np.ascontiguousarray(e2_b),
        "eo_W": np.ascontiguousarray(eo_W),
        "eo_b": np.ascontiguousarray(eo_b),
        "head_W0": np.ascontiguousarray(head_W[0, :, :LATENT]),
        "head_W1": np.ascontiguousarray(head_W[1, :, :RDIM]),
        "head_b0": np.ascontiguousarray(head_b[0, :LATENT].reshape(4, 128)),
        "head_b1": np.ascontiguousarray(head_b[1, :RDIM].reshape(RDIM, 1)),
        "c_ones8": np.ones((8, 1), np.float32),
        "c_ones18": np.ones((1, 8), np.float32),
        "c_ones1x128": np.ones((1, 128), np.float32),
        "c_sel": sel_np,
    }
    in_maps = []
    for c in range(n_cores):
        m = dict(shared)
        m["zT"] = np.ascontiguousarray(zT[:, c * rows:(c + 1) * rows])
        m["aT"] = np.ascontiguousarray(aT[:, c * rows:(c + 1) * rows])
        in_maps.append(m)
    return in_maps


def kernel(z, a, pre_W, pre_b, g1_W, g1_b, g2_W, g2_b,
           e1_W, e1_b, e2_W, e2_b, eo_W, eo_b, head_W, head_b):
    from concourse.bass_utils import run_bass_kernel_spmd

    args = [np.asarray(x, dtype=np.float32) for x in
            (z, a, pre_W, pre_b, g1_W, g1_b, g2_W, g2_b,
             e1_W, e1_b, e2_W, e2_b, eo_W, eo_b, head_W, head_b)]
    key = ("full", ROWS, N_CORES)
    if key not in _cache:
        _cache[key] = _build(ROWS, N_CORES)
    nc = _cache[key]
    in_maps = _host_inputs(*args, rows=ROWS, n_cores=N_CORES)
    res = run_bass_kernel_spmd(nc, in_maps, core_ids=list(range(N_CORES)))

    next_z = np.empty((N, LATENT), np.float32)
    reward = np.empty((N, RDIM), np.float32)
    w_dyn = np.empty((N, E), np.float32)
    w_rew = np.empty((N, E), np.float32)
    for c in range(N_CORES):
        r = res.results[c]
        sl = slice(c * ROWS, (c + 1) * ROWS)
        next_z[sl] = r["nzT"].T
        reward[sl] = r["rewT"].T
        w_dyn[sl] = r["wdynT"].T
        w_rew[sl] = r["wrewT"].T
    next_z += args[0]          # residual added on host
    return next_z, reward, w_dyn, w_rew


# revision 37
# speedup vs baseline: 1.0571x; 1.0571x over previous
"""Trainium2 Bass kernel: MoE transition/reward model, data-parallel on 8 cores.

Layout: all on-device compute is feature-major (features on SBUF partitions,
batch rows on the free dim); the host transposes inputs/outputs, which keeps
every matmul in the natural lhsT=weight / rhs=activation form with zero
on-device transposes. Matmuls run in float32r (full-rate fp32 mode).
"""
import numpy as np

N = 16384
LATENT = 512
ACTION = 64
HIDDEN = 1024
E = 8
RDIM = 101
N_CORES = 8
ROWS = N // N_CORES            # rows per core
PASS_ROWS = 1024               # rows per weight-streaming pass
CHUNK = 512                    # matmul moving free dim
KD = LATENT // 128             # 4 z k-chunks
KH = HIDDEN // 128             # 8 hidden k-chunks

_cache = {}


def _emit(nc, tc, io, rows, ctx):
    import concourse.mybir as mybir
    from concourse.mybir import AluOpType as Op
    from concourse.mybir import ActivationFunctionType as Act
    f32 = mybir.dt.float32
    f32r = mybir.dt.float32r

    n_pass = rows // PASS_ROWS if rows >= PASS_ROWS else 1
    pass_rows = min(rows, PASS_ROWS)
    n_chunk = pass_rows // CHUNK

    Pc = ctx.enter_context(tc.tile_pool(name="consts", bufs=1))
    P1 = ctx.enter_context(tc.tile_pool(name="p1", bufs=1))
    Pws = ctx.enter_context(tc.tile_pool(name="ws", bufs=5))
    Ptmp = ctx.enter_context(tc.tile_pool(name="tmp", bufs=4))
    Pbc = ctx.enter_context(tc.tile_pool(name="bc", bufs=4))
    Pout = ctx.enter_context(tc.tile_pool(name="out", bufs=2))
    Pps = ctx.enter_context(tc.tile_pool(name="ps", bufs=8, space="PSUM"))

    # ---- constants / small weights (loaded once) ----
    g2w = Pc.tile([128, KH * 16], f32r, name="g2w")
    nc.gpsimd.dma_start(g2w[:], io["g2_W"].rearrange("(k p) m -> p k m", p=128))
    ones8 = Pc.tile([8, 1], f32r, name="ones8")
    nc.gpsimd.dma_start(ones8[:], io["c_ones8"][:])
    ones18 = Pc.tile([1, 8], f32r, name="ones18")
    nc.gpsimd.dma_start(ones18[:], io["c_ones18"][:])
    ones1x128 = Pc.tile([1, 128], f32r, name="ones1x128")
    nc.gpsimd.dma_start(ones1x128[:], io["c_ones1x128"][:])
    sel = Pc.tile([8, E * 128], f32r, name="sel")
    nc.gpsimd.dma_start(sel[:], io["c_sel"][:])
    ebt = Pc.tile([8, LATENT], f32r, name="ebt")
    nc.gpsimd.dma_start(ebt[:], io["eo_b"][:])
    preb = Pc.tile([128, 8], f32, name="preb")
    nc.gpsimd.dma_start(preb[:], io["pre_b"].rearrange("o p -> p o"))
    g1b = Pc.tile([128, 8], f32, name="g1b")
    nc.gpsimd.dma_start(g1b[:], io["g1_b"].rearrange("o p -> p o"))
    g2bd = Pc.tile([8, 1], f32, name="g2bd")
    nc.gpsimd.dma_start(g2bd[:], io["g2_b"][0:8, :])
    g2br = Pc.tile([8, 1], f32, name="g2br")
    nc.gpsimd.dma_start(g2br[:], io["g2_b"][8:16, :])
    hb0 = Pc.tile([128, 4], f32, name="hb0")
    nc.gpsimd.dma_start(hb0[:], io["head_b0"].rearrange("o p -> p o"))
    hb1 = Pc.tile([RDIM, 1], f32, name="hb1")
    nc.gpsimd.dma_start(hb1[:], io["head_b1"].rearrange("o p -> p o"))

    e1W = io["e1_W"]   # [E, H, H] f32r
    e2W = io["e2_W"]
    eoW = io["eo_W"]   # [E, H, LATENT]

    for p in range(n_pass):
        ps = p * pass_rows

        # ================= phase A: inputs, pre_proj, gate =================
        # zt alternates tags by pass parity so the next pass's z can prefetch
        # while this pass's accumulators still occupy "ztacc".
        zt = P1.tile([128, KD * pass_rows], f32r,
                     tag=("ztacc" if p % 2 == 0 else "m1b"), name=f"zt{p}")
        for k in range(KD):
            nc.sync.dma_start(zt[:, k * pass_rows:(k + 1) * pass_rows],
                              io["zT"][k * 128:(k + 1) * 128, ps:ps + pass_rows])
        at = P1.tile([64, pass_rows], f32r, tag="at", name=f"at{p}")
        nc.sync.dma_start(at[:], io["aT"][:, ps:ps + pass_rows])
        pw = P1.tile([128, 5 * 1024], f32r, tag="A1", name=f"pw{p}")
        for k in range(5):
            kp = 128 if k < 4 else 64
            nc.sync.dma_start(pw[:kp, k * 1024:(k + 1) * 1024],
                              io["pre_W"][k * 128:k * 128 + kp, :])
        g1w = P1.tile([128, KH * 1024], f32r, tag="A2", name=f"g1w{p}")
        ht = P1.tile([128, KH * pass_rows], f32r, tag="ht", name=f"ht{p}")

        # pre_proj: h = [z;a] @ pre_W + pre_b   (no relu)
        for og in range(4):
            pss = {}
            for o2 in range(2):
                for r in range(n_chunk):
                    pss[o2, r] = Pps.tile([128, CHUNK], f32, tag="ps",
                                          name=f"pp{p}_{og}_{o2}_{r}")
            for k in range(5):
                kp = 128 if k < 4 else 64
                for o2 in range(2):
                    o = og * 2 + o2
                    for r in range(n_chunk):
                        if k < 4:
                            rhs = zt[:, k * pass_rows + r * CHUNK:
                                     k * pass_rows + r * CHUNK + CHUNK]
                        else:
                            rhs = at[:, r * CHUNK:r * CHUNK + CHUNK]
                        nc.tensor.matmul(pss[o2, r][:],
                                         pw[:kp, k * 1024 + o * 128:k * 1024 + o * 128 + 128],
                                         rhs, start=(k == 0), stop=(k == 4))
            for o2 in range(2):
                o = og * 2 + o2
                for r in range(n_chunk):
                    nc.vector.tensor_scalar_add(
                        ht[:, o * pass_rows + r * CHUNK:o * pass_rows + r * CHUNK + CHUNK],
                        pss[o2, r][:], preb[:, o:o + 1])
            if og == 0:
                # emitted after the first pre og so the SP DMA ring serves
                # zt/at/pw (needed first) before this 4MB transfer
                nc.sync.dma_start(g1w[:],
                                  io["g1_W"].rearrange("(k p) m -> p k m", p=128))

        # gate MLP: g1 with both row-chunks inside each og group (longer PE
        # bursts; copybacks hide). m1 per row-chunk on separate tags.
        wd = P1.tile([8, pass_rows], f32r, tag="at", name=f"wd{p}")
        wr = P1.tile([8, pass_rows], f32r, tag="wr", name=f"wr{p}")
        m1s = []
        for r in range(n_chunk):
            m1s.append(P1.tile([128, KH * CHUNK], f32r,
                               tag=("accr" if r == 0 else "m1b"),
                               name=f"m1_{p}_{r}"))
        for og in range(4):
            pss = {}
            for o2 in range(2):
                for r in range(n_chunk):
                    pss[o2, r] = Pps.tile([128, CHUNK], f32, tag="ps",
                                          name=f"pg{p}_{og}_{o2}_{r}")
            for k in range(KH):
                for o2 in range(2):
                    o = og * 2 + o2
                    for r in range(n_chunk):
                        nc.tensor.matmul(
                            pss[o2, r][:],
                            g1w[:, k * 1024 + o * 128:k * 1024 + o * 128 + 128],
                            ht[:, k * pass_rows + r * CHUNK:k * pass_rows + r * CHUNK + CHUNK],
                            start=(k == 0), stop=(k == KH - 1))
            for o2 in range(2):
                o = og * 2 + o2
                for r in range(n_chunk):
                    nc.vector.tensor_scalar(
                        m1s[r][:, o * CHUNK:(o + 1) * CHUNK],
                        pss[o2, r][:], g1b[:, o:o + 1], 0.0,
                        op0=Op.add, op1=Op.max)
        # Softmax with deferred normalization: wd/wr hold UNNORMALIZED exp;
        # 1/sum lands in recd/recr and is applied to the head outputs in
        # phase C (a per-column scale commutes through the head matmul).
        # This keeps the slow 1-partition reciprocal off the PE critical path.
        rec_all = P1.tile([1, 2 * pass_rows], f32r, tag="rec", name=f"rec{p}")
        recd = rec_all[:, 0:pass_rows]
        recr = rec_all[:, pass_rows:2 * pass_rows]
        gcfg = [(r, gi) for r in range(n_chunk) for gi in range(2)]
        gmeta = {0: ((0, 8), g2bd, wd, recd, io["wdynT"]),
                 1: ((8, 16), g2br, wr, recr, io["wrewT"])}
        pgs = {}
        for r, gi in gcfg:
            gslice = gmeta[gi][0]
            pg = Pps.tile([8, CHUNK], f32, tag="ps", name=f"pgg{p}_{r}_{gi}")
            for k in range(KH):
                nc.tensor.matmul(
                    pg[:], g2w[:, k * 16 + gslice[0]:k * 16 + gslice[1]],
                    m1s[r][:, k * CHUNK:(k + 1) * CHUNK],
                    start=(k == 0), stop=(k == KH - 1))
            pgs[r, gi] = pg
            nc.scalar.activation(gmeta[gi][2][:, r * CHUNK:(r + 1) * CHUNK],
                                 pg[:], Act.Exp, bias=gmeta[gi][1][:, 0:1])
        scps = {}
        for r, gi in gcfg:
            wt = gmeta[gi][2]
            psums = Pps.tile([1, CHUNK], f32, tag="ps", name=f"psum{p}_{r}_{gi}")
            nc.tensor.matmul(psums[:], ones8[:],
                             wt[:, r * CHUNK:(r + 1) * CHUNK],
                             start=True, stop=True)
            # fast PSUM->SBUF copy on ACT releases the bank; the slow
            # 1-partition reciprocal then runs off the critical path
            scp = Ptmp.tile([1, CHUNK], f32, tag="tmp", name=f"scp{p}_{r}_{gi}")
            nc.scalar.activation(scp[:], psums[:], Act.Copy)
            scps[r, gi] = scp

        # ================= phase B: experts =================
        accd = P1.tile([128, KD * pass_rows], f32r, tag="ztacc", name=f"accd{p}")
        accr = P1.tile([128, KD * pass_rows], f32r, tag="accr", name=f"accr{p}")
        # init acc with gate-weighted eo bias: acc[d, n] = sum_e eo_b[e, d] w[e, n]
        for r in range(n_chunk):
            for d in range(KD):
                pi = Pps.tile([128, CHUNK], f32, tag="ps", name=f"pid{p}_{r}_{d}")
                nc.tensor.matmul(pi[:], ebt[:, d * 128:(d + 1) * 128],
                                 wd[:, r * CHUNK:(r + 1) * CHUNK],
                                 start=True, stop=True)
                nc.scalar.activation(
                    accd[:, d * pass_rows + r * CHUNK:d * pass_rows + r * CHUNK + CHUNK],
                    pi[:], Act.Copy)
                pi2 = Pps.tile([128, CHUNK], f32, tag="ps", name=f"pir{p}_{r}_{d}")
                nc.tensor.matmul(pi2[:], ebt[:, d * 128:(d + 1) * 128],
                                 wr[:, r * CHUNK:(r + 1) * CHUNK],
                                 start=True, stop=True)
                nc.vector.tensor_copy(
                    accr[:, d * pass_rows + r * CHUNK:d * pass_rows + r * CHUNK + CHUNK],
                    pi2[:])

        # reciprocals after the init block, as cheap approx ops (sums are
        # well-conditioned; 18 bits is far beyond the f32r noise floor)
        for r, gi in gcfg:
            rec = gmeta[gi][3]
            rf = Pbc.tile([1, CHUNK], f32, tag="bc", name=f"rf{p}_{r}_{gi}")
            nc.vector.reciprocal_approx_fast(rf[:], scps[r, gi][:])
            nc.vector.tensor_copy(rec[:, r * CHUNK:(r + 1) * CHUNK], rf[:])

        for e in range(E):
            # broadcast gate weights across partitions via one-hot matmul
            bc = {}
            for gi, wt in ((0, wd), (1, wr)):
                for r in range(n_chunk):
                    pb = Pps.tile([128, CHUNK], f32, tag="ps",
                                  name=f"pb{p}_{e}_{gi}_{r}")
                    nc.tensor.matmul(pb[:], sel[:, e * 128:(e + 1) * 128],
                                     wt[:, r * CHUNK:(r + 1) * CHUNK],
                                     start=True, stop=True)
                    t = Pbc.tile([128, CHUNK], f32, tag="bc",
                                 name=f"bc{p}_{e}_{gi}_{r}")
                    nc.vector.tensor_copy(t[:], pb[:])
                    bc[gi, r] = t
            e1b = Pws.tile([128, 8], f32, tag="eb1", name=f"e1b{p}_{e}")
            nc.gpsimd.dma_start(e1b[:], io["e1_b"][e].rearrange("(o p) -> p o", p=128))
            e2b = Pws.tile([128, 8], f32, tag="eb2", name=f"e2b{p}_{e}")
            nc.gpsimd.dma_start(e2b[:], io["e2_b"][e].rearrange("(o p) -> p o", p=128))

            # x1 = relu(e1_W[e].T @ h + e1_b[e])
            x1 = P1.tile([128, KH * pass_rows], f32r, tag="A1", name=f"x1_{p}_{e}")
            for og in range(4):
                wsth = []
                for hh in range(2):
                    t = Pws.tile([128, 4 * 256], f32r, tag="ws",
                                 name=f"w1_{p}_{e}_{og}_{hh}")
                    nc.sync.dma_start(
                        t[:], e1W[e, hh * 512:(hh + 1) * 512,
                                  og * 256:(og + 1) * 256]
                        .rearrange("(k p) m -> p k m", p=128))
                    wsth.append(t)
                pss = {}
                for o2 in range(2):
                    for r in range(n_chunk):
                        pss[o2, r] = Pps.tile([128, CHUNK], f32, tag="ps",
                                              name=f"px1{p}_{e}_{og}_{o2}_{r}")
                for k in range(KH):
                    for o2 in range(2):
                        for r in range(n_chunk):
                            nc.tensor.matmul(
                                pss[o2, r][:],
                                wsth[k // 4][:, (k % 4) * 256 + o2 * 128:(k % 4) * 256 + o2 * 128 + 128],
                                ht[:, k * pass_rows + r * CHUNK:k * pass_rows + r * CHUNK + CHUNK],
                                start=(k == 0), stop=(k == KH - 1))
                for o2 in range(2):
                    o = og * 2 + o2
                    for r in range(n_chunk):
                        nc.scalar.activation(
                            x1[:, o * pass_rows + r * CHUNK:o * pass_rows + r * CHUNK + CHUNK],
                            pss[o2, r][:], Act.Relu, bias=e1b[:, o:o + 1])

            if e == 0:
                # normalized gate-weight outputs; emitted an expert-stage in
                # so the reciprocals (slow, DVE) are long done
                for r, gi in gcfg:
                    wt, rec, wout = gmeta[gi][2], gmeta[gi][3], gmeta[gi][4]
                    pb8 = Pps.tile([8, CHUNK], f32, tag="ps",
                                   name=f"pb8{p}_{r}_{gi}")
                    nc.tensor.matmul(pb8[:], ones18[:],
                                     rec[:, r * CHUNK:(r + 1) * CHUNK],
                                     start=True, stop=True)
                    wn = Pout.tile([8, CHUNK], f32r, tag="out",
                                   name=f"wn{p}_{r}_{gi}")
                    nc.vector.tensor_tensor(wn[:],
                                            wt[:, r * CHUNK:(r + 1) * CHUNK],
                                            pb8[:], op=Op.mult)
                    nc.scalar.dma_start(
                        wout[:, ps + r * CHUNK:ps + (r + 1) * CHUNK], wn[:])

            # x2 = relu(e2_W[e].T @ x1 + e2_b[e])
            x2 = P1.tile([128, KH * pass_rows], f32r, tag="A2", name=f"x2_{p}_{e}")
            for og in range(4):
                wsth = []
                for hh in range(2):
                    t = Pws.tile([128, 4 * 256], f32r, tag="ws",
                                 name=f"w2_{p}_{e}_{og}_{hh}")
                    nc.sync.dma_start(
                        t[:], e2W[e, hh * 512:(hh + 1) * 512,
                                  og * 256:(og + 1) * 256]
                        .rearrange("(k p) m -> p k m", p=128))
                    wsth.append(t)
                pss = {}
                for o2 in range(2):
                    for r in range(n_chunk):
                        pss[o2, r] = Pps.tile([128, CHUNK], f32, tag="ps",
                                              name=f"px2{p}_{e}_{og}_{o2}_{r}")
                for k in range(KH):
                    for o2 in range(2):
                        for r in range(n_chunk):
                            nc.tensor.matmul(
                                pss[o2, r][:],
                                wsth[k // 4][:, (k % 4) * 256 + o2 * 128:(k % 4) * 256 + o2 * 128 + 128],
                                x1[:, k * pass_rows + r * CHUNK:k * pass_rows + r * CHUNK + CHUNK],
                                start=(k == 0), stop=(k == KH - 1))
                for o2 in range(2):
                    o = og * 2 + o2
                    for r in range(n_chunk):
                        nc.vector.tensor_scalar(
                            x2[:, o * pass_rows + r * CHUNK:o * pass_rows + r * CHUNK + CHUNK],
                            pss[o2, r][:], e2b[:, o:o + 1], 0.0,
                            op0=Op.add, op1=Op.max)

            # feats = eo_W[e].T @ x2; acc += w[e] * feats (both gates)
            for dg in range(2):
                wsth = []
                for hh in range(2):
                    t = Pws.tile([128, 4 * 256], f32r, tag="ws",
                                 name=f"wo_{p}_{e}_{dg}_{hh}")
                    nc.sync.dma_start(
                        t[:], eoW[e, hh * 512:(hh + 1) * 512,
                                  dg * 256:(dg + 1) * 256]
                        .rearrange("(k p) m -> p k m", p=128))
                    wsth.append(t)
                pfs = {}
                for d2 in range(2):
                    for r in range(n_chunk):
                        pfs[d2, r] = Pps.tile([128, CHUNK], f32, tag="ps",
                                              name=f"pf{p}_{e}_{dg}_{d2}_{r}")
                for k in range(KH):
                    for d2 in range(2):
                        for r in range(n_chunk):
                            nc.tensor.matmul(
                                pfs[d2, r][:],
                                wsth[k // 4][:, (k % 4) * 256 + d2 * 128:(k % 4) * 256 + d2 * 128 + 128],
                                x2[:, k * pass_rows + r * CHUNK:k * pass_rows + r * CHUNK + CHUNK],
                                start=(k == 0), stop=(k == KH - 1))
                for d2 in range(2):
                    d = dg * 2 + d2
                    for r in range(n_chunk):
                        c0 = d * pass_rows + r * CHUNK
                        tmp = Ptmp.tile([128, CHUNK], f32, tag="tmp",
                                        name=f"td{p}_{e}_{dg}_{d2}_{r}")
                        nc.vector.tensor_tensor(tmp[:], pfs[d2, r][:],
                                                bc[0, r][:], op=Op.mult)
                        # acc += tmp on GpSimd: keeps DVE free for the PSUM
                        # reads, so PSUM banks recycle fast at eo boundaries
                        nc.gpsimd.tensor_add(accd[:, c0:c0 + CHUNK],
                                             accd[:, c0:c0 + CHUNK], tmp[:])
                        tmp2 = Ptmp.tile([128, CHUNK], f32, tag="tmp",
                                         name=f"tr{p}_{e}_{dg}_{d2}_{r}")
                        nc.vector.tensor_tensor(tmp2[:], pfs[d2, r][:],
                                                bc[1, r][:], op=Op.mult)
                        nc.gpsimd.tensor_add(accr[:, c0:c0 + CHUNK],
                                             accr[:, c0:c0 + CHUNK], tmp2[:])

        # ================= phase C: fused heads =================
        hw0h = []
        for hh in range(2):
            t = Pws.tile([128, 2 * 512], f32r, tag="ws", name=f"hw0_{p}_{hh}")
            nc.sync.dma_start(
                t[:], io["head_W0"][hh * 256:(hh + 1) * 256, :]
                .rearrange("(k p) m -> p k m", p=128))
            hw0h.append(t)
        hw1 = Pws.tile([128, 4 * RDIM], f32r, tag="ws", name=f"hw1_{p}")
        nc.sync.dma_start(hw1[:], io["head_W1"].rearrange("(k p) m -> p k m", p=128))
        for r in range(n_chunk):
            # broadcast 1/sum to 128 partitions for the deferred normalization
            rb = {}
            for gi, rec in ((0, recd), (1, recr)):
                pbc = Pps.tile([128, CHUNK], f32, tag="ps", name=f"pbc{p}_{r}_{gi}")
                nc.tensor.matmul(pbc[:], ones1x128[:],
                                 rec[:, r * CHUNK:(r + 1) * CHUNK],
                                 start=True, stop=True)
                t = Pbc.tile([128, CHUNK], f32, tag="bc", name=f"rb{p}_{r}_{gi}")
                nc.vector.tensor_copy(t[:], pbc[:])
                rb[gi] = t
            for d in range(4):
                ph = Pps.tile([128, CHUNK], f32, tag="ps", name=f"ph{p}_{r}_{d}")
                for k in range(KD):
                    nc.tensor.matmul(
                        ph[:], hw0h[k // 2][:, (k % 2) * 512 + d * 128:(k % 2) * 512 + d * 128 + 128],
                        accd[:, k * pass_rows + r * CHUNK:k * pass_rows + r * CHUNK + CHUNK],
                        start=(k == 0), stop=(k == KD - 1))
                t1 = Ptmp.tile([128, CHUNK], f32, tag="tmp", name=f"th{p}_{r}_{d}")
                nc.vector.tensor_tensor(t1[:], ph[:], rb[0][:], op=Op.mult)
                ot = Pout.tile([128, CHUNK], f32, tag="out", name=f"ot{p}_{r}_{d}")
                nc.vector.tensor_scalar_add(ot[:], t1[:], hb0[:, d:d + 1])
                nc.scalar.dma_start(
                    io["nzT"][d * 128:(d + 1) * 128, ps + r * CHUNK:ps + r * CHUNK + CHUNK],
                    ot[:])
            pr = Pps.tile([RDIM, CHUNK], f32, tag="ps", name=f"prew{p}_{r}")
            for k in range(KD):
                nc.tensor.matmul(
                    pr[:], hw1[:, k * RDIM:(k + 1) * RDIM],
                    accr[:, k * pass_rows + r * CHUNK:k * pass_rows + r * CHUNK + CHUNK],
                    start=(k == 0), stop=(k == KD - 1))
            t2 = Ptmp.tile([RDIM, CHUNK], f32, tag="tmp", name=f"thr{p}_{r}")
            nc.vector.tensor_tensor(t2[:], pr[:], rb[1][0:RDIM, :], op=Op.mult)
            orw = Pout.tile([RDIM, CHUNK], f32, tag="out", name=f"orw{p}_{r}")
            nc.vector.tensor_scalar_add(orw[:], t2[:], hb1[:, 0:1])
            nc.scalar.dma_start(
                io["rewT"][:, ps + r * CHUNK:ps + r * CHUNK + CHUNK], orw[:])


def _build(rows, n_cores):
    import concourse.bacc as bacc
    import concourse.tile as tile
    import concourse.mybir as mybir
    f32 = mybir.dt.float32
    f32r = mybir.dt.float32r

    nc = bacc.Bacc("TRN2", target_bir_lowering=False, debug=False,
                   num_devices=n_cores)
    io = {}

    def inp(name, shape, dt):
        io[name] = nc.dram_tensor(name, shape, dt, kind="ExternalInput").ap()

    def outp(name, shape, dt):
        io[name] = nc.dram_tensor(name, shape, dt, kind="ExternalOutput").ap()

    inp("zT", [LATENT, rows], f32r)
    inp("aT", [ACTION, rows], f32r)
    inp("pre_W", [LATENT + ACTION, HIDDEN], f32r)
    inp("pre_b", [8, 128], f32)
    inp("g1_W", [HIDDEN, HIDDEN], f32r)
    inp("g1_b", [8, 128], f32)
    inp("g2_W", [HIDDEN, 2 * E], f32r)
    inp("g2_b", [2 * E, 1], f32)
    inp("e1_W", [E, HIDDEN, HIDDEN], f32r)
    inp("e1_b", [E, HIDDEN], f32)
    inp("e2_W", [E, HIDDEN, HIDDEN], f32r)
    inp("e2_b", [E, HIDDEN], f32)
    inp("eo_W", [E, HIDDEN, LATENT], f32r)
    inp("eo_b", [E, LATENT], f32r)
    inp("head_W0", [LATENT, LATENT], f32r)
    inp("head_W1", [LATENT, RDIM], f32r)
    inp("head_b0", [4, 128], f32)
    inp("head_b1", [RDIM, 1], f32)
    inp("c_ones8", [8, 1], f32r)
    inp("c_ones18", [1, 8], f32r)
    inp("c_ones1x128", [1, 128], f32r)
    inp("c_sel", [8, E * 128], f32r)

    outp("nzT", [LATENT, rows], f32)
    outp("rewT", [RDIM, rows], f32)
    outp("wdynT", [E, rows], f32r)
    outp("wrewT", [E, rows], f32r)

    from contextlib import ExitStack
    with tile.TileContext(nc) as tc:
        with ExitStack() as ctx:
            _emit(nc, tc, io, rows, ctx)
    nc.compile()
    return nc


def _host_inputs(z, a, pre_W, pre_b, g1_W, g1_b, g2_W, g2_b,
                 e1_W, e1_b, e2_W, e2_b, eo_W, eo_b, head_W, head_b,
                 rows, n_cores):
    """Build per-core in_maps (host-side transposes/shaping)."""
    zT = np.ascontiguousarray(z.T)
    aT = np.ascontiguousarray(a.T)
    sel_np = np.zeros((8, E * 128), np.float32)
    for e in range(E):
        sel_np[e, e * 128:(e + 1) * 128] = 1.0
    shared = {
        "pre_W": np.ascontiguousarray(pre_W),
        "pre_b": np.ascontiguousarray(pre_b.reshape(8, 128)),
        "g1_W": np.ascontiguousarray(g1_W),
        "g1_b": np.ascontiguousarray(g1_b.reshape(8, 128)),
        "g2_W": np.ascontiguousarray(g2_W),
        "g2_b": np.ascontiguousarray(g2_b.reshape(2 * E, 1)),
        "e1_W": np.ascontiguousarray(e1_W),
        "e1_b": np.ascontiguousarray(e1_b),
        "e2_W": np.ascontiguousarray(e2_W),
        "e2_b": np.ascontiguousarray(e2_b),
        "eo_W": np.ascontiguousarray(eo_W),
        "eo_b": np.ascontiguousarray(eo_b),
        "head_W0": np.ascontiguousarray(head_W[0, :, :LATENT]),
        "head_W1": np.ascontiguousarray(head_W[1, :, :RDIM]),
        "head_b0": np.ascontiguousarray(head_b[0, :LATENT].reshape(4, 128)),
        "head_b1": np.ascontiguousarray(head_b[1, :RDIM].reshape(RDIM, 1)),
        "c_ones8": np.ones((8, 1), np.float32),
        "c_ones18": np.ones((1, 8), np.float32),
        "c_ones1x128": np.ones((1, 128), np.float32),
        "c_sel": sel_np,
    }
    in_maps = []
    for c in range(n_cores):
        m = dict(shared)
        m["zT"] = np.ascontiguousarray(zT[:, c * rows:(c + 1) * rows])
        m["aT"] = np.ascontiguousarray(aT[:, c * rows:(c + 1) * rows])
        in_maps.append(m)
    return in_maps


def kernel(z, a, pre_W, pre_b, g1_W, g1_b, g2_W, g2_b,
           e1_W, e1_b, e2_W, e2_b, eo_W, eo_b, head_W, head_b):
    from concourse.bass_utils import run_bass_kernel_spmd

    args = [np.asarray(x, dtype=np.float32) for x in
            (z, a, pre_W, pre_b, g1_W, g1_b, g2_W, g2_b,
             e1_W, e1_b, e2_W, e2_b, eo_W, eo_b, head_W, head_b)]
    key = ("full", ROWS, N_CORES)
    if key not in _cache:
        _cache[key] = _build(ROWS, N_CORES)
    nc = _cache[key]
    in_maps = _host_inputs(*args, rows=ROWS, n_cores=N_CORES)
    res = run_bass_kernel_spmd(nc, in_maps, core_ids=list(range(N_CORES)))

    next_z = np.empty((N, LATENT), np.float32)
    reward = np.empty((N, RDIM), np.float32)
    w_dyn = np.empty((N, E), np.float32)
    w_rew = np.empty((N, E), np.float32)
    for c in range(N_CORES):
        r = res.results[c]
        sl = slice(c * ROWS, (c + 1) * ROWS)
        next_z[sl] = r["nzT"].T
        reward[sl] = r["rewT"].T
        w_dyn[sl] = r["wdynT"].T
        w_rew[sl] = r["wrewT"].T
    next_z += args[0]          # residual added on host
    return next_z, reward, w_dyn, w_rew


# revision 38
# speedup vs baseline: 1.0627x; 1.0053x over previous
"""Trainium2 Bass kernel: MoE transition/reward model, data-parallel on 8 cores.

Layout: all on-device compute is feature-major (features on SBUF partitions,
batch rows on the free dim); the host transposes inputs/outputs, which keeps
every matmul in the natural lhsT=weight / rhs=activation form with zero
on-device transposes. Matmuls run in float32r (full-rate fp32 mode).
"""
import numpy as np

N = 16384
LATENT = 512
ACTION = 64
HIDDEN = 1024
E = 8
RDIM = 101
N_CORES = 8
ROWS = N // N_CORES            # rows per core
PASS_ROWS = 1024               # rows per weight-streaming pass
CHUNK = 512                    # matmul moving free dim
KD = LATENT // 128             # 4 z k-chunks
KH = HIDDEN // 128             # 8 hidden k-chunks

_cache = {}


def _emit(nc, tc, io, rows, ctx):
    import concourse.mybir as mybir
    from concourse.mybir import AluOpType as Op
    from concourse.mybir import ActivationFunctionType as Act
    f32 = mybir.dt.float32
    f32r = mybir.dt.float32r

    n_pass = rows // PASS_ROWS if rows >= PASS_ROWS else 1
    pass_rows = min(rows, PASS_ROWS)
    n_chunk = pass_rows // CHUNK

    Pc = ctx.enter_context(tc.tile_pool(name="consts", bufs=1))
    P1 = ctx.enter_context(tc.tile_pool(name="p1", bufs=1))
    Pws = ctx.enter_context(tc.tile_pool(name="ws", bufs=5))
    Ptmp = ctx.enter_context(tc.tile_pool(name="tmp", bufs=4))
    Pbc = ctx.enter_context(tc.tile_pool(name="bc", bufs=4))
    Pout = ctx.enter_context(tc.tile_pool(name="out", bufs=2))
    Pps = ctx.enter_context(tc.tile_pool(name="ps", bufs=8, space="PSUM"))

    # ---- constants / small weights (loaded once) ----
    g2w = Pc.tile([128, KH * 16], f32r, name="g2w")
    nc.gpsimd.dma_start(g2w[:], io["g2_W"].rearrange("(k p) m -> p k m", p=128))
    ones8 = Pc.tile([8, 1], f32r, name="ones8")
    nc.gpsimd.dma_start(ones8[:], io["c_ones8"][:])
    ones18 = Pc.tile([1, 8], f32r, name="ones18")
    nc.gpsimd.dma_start(ones18[:], io["c_ones18"][:])
    ones1x128 = Pc.tile([1, 128], f32r, name="ones1x128")
    nc.gpsimd.dma_start(ones1x128[:], io["c_ones1x128"][:])
    sel = Pc.tile([8, E * 128], f32r, name="sel")
    nc.gpsimd.dma_start(sel[:], io["c_sel"][:])
    ebt = Pc.tile([8, LATENT], f32r, name="ebt")
    nc.gpsimd.dma_start(ebt[:], io["eo_b"][:])
    preb = Pc.tile([128, 8], f32, name="preb")
    nc.gpsimd.dma_start(preb[:], io["pre_b"].rearrange("o p -> p o"))
    g1b = Pc.tile([128, 8], f32, name="g1b")
    nc.gpsimd.dma_start(g1b[:], io["g1_b"].rearrange("o p -> p o"))
    g2bd = Pc.tile([8, 1], f32, name="g2bd")
    nc.gpsimd.dma_start(g2bd[:], io["g2_b"][0:8, :])
    g2br = Pc.tile([8, 1], f32, name="g2br")
    nc.gpsimd.dma_start(g2br[:], io["g2_b"][8:16, :])
    hb0 = Pc.tile([128, 4], f32, name="hb0")
    nc.gpsimd.dma_start(hb0[:], io["head_b0"].rearrange("o p -> p o"))
    hb1 = Pc.tile([RDIM, 1], f32, name="hb1")
    nc.gpsimd.dma_start(hb1[:], io["head_b1"].rearrange("o p -> p o"))

    e1W = io["e1_W"]   # [E, H, H] f32r
    e2W = io["e2_W"]
    eoW = io["eo_W"]   # [E, H, LATENT]

    for p in range(n_pass):
        ps = p * pass_rows

        # ================= phase A: inputs, pre_proj, gate =================
        # zt alternates tags by pass parity so the next pass's z can prefetch
        # while this pass's accumulators still occupy "ztacc".
        zt = P1.tile([128, KD * pass_rows], f32r,
                     tag=("ztacc" if p % 2 == 0 else "m1b"), name=f"zt{p}")
        nc.sync.dma_start(zt[:], io["zT"][:, ps:ps + pass_rows]
                          .rearrange("(k p) n -> p k n", p=128))
        at = P1.tile([64, pass_rows], f32r, tag="at", name=f"at{p}")
        nc.sync.dma_start(at[:], io["aT"][:, ps:ps + pass_rows])
        pw = P1.tile([128, 5 * 1024], f32r, tag="A1", name=f"pw{p}")
        nc.sync.dma_start(pw[:, 0:4 * 1024],
                          io["pre_W"][0:512, :].rearrange("(k p) m -> p k m", p=128))
        nc.sync.dma_start(pw[:64, 4 * 1024:5 * 1024], io["pre_W"][512:576, :])
        g1w = P1.tile([128, KH * 1024], f32r, tag="A2", name=f"g1w{p}")
        ht = P1.tile([128, KH * pass_rows], f32r, tag="ht", name=f"ht{p}")

        # pre_proj: h = [z;a] @ pre_W + pre_b   (no relu)
        for og in range(4):
            pss = {}
            for o2 in range(2):
                for r in range(n_chunk):
                    pss[o2, r] = Pps.tile([128, CHUNK], f32, tag="ps",
                                          name=f"pp{p}_{og}_{o2}_{r}")
            for k in range(5):
                kp = 128 if k < 4 else 64
                for o2 in range(2):
                    o = og * 2 + o2
                    for r in range(n_chunk):
                        if k < 4:
                            rhs = zt[:, k * pass_rows + r * CHUNK:
                                     k * pass_rows + r * CHUNK + CHUNK]
                        else:
                            rhs = at[:, r * CHUNK:r * CHUNK + CHUNK]
                        nc.tensor.matmul(pss[o2, r][:],
                                         pw[:kp, k * 1024 + o * 128:k * 1024 + o * 128 + 128],
                                         rhs, start=(k == 0), stop=(k == 4))
            for o2 in range(2):
                o = og * 2 + o2
                for r in range(n_chunk):
                    nc.vector.tensor_scalar_add(
                        ht[:, o * pass_rows + r * CHUNK:o * pass_rows + r * CHUNK + CHUNK],
                        pss[o2, r][:], preb[:, o:o + 1])
            if og == 0:
                # emitted after the first pre og so the SP DMA ring serves
                # zt/at/pw (needed first) before this 4MB transfer
                nc.sync.dma_start(g1w[:],
                                  io["g1_W"].rearrange("(k p) m -> p k m", p=128))

        # gate MLP: g1 with both row-chunks inside each og group (longer PE
        # bursts; copybacks hide). m1 per row-chunk on separate tags.
        wd = P1.tile([8, pass_rows], f32r, tag="at", name=f"wd{p}")
        wr = P1.tile([8, pass_rows], f32r, tag="wr", name=f"wr{p}")
        m1s = []
        for r in range(n_chunk):
            m1s.append(P1.tile([128, KH * CHUNK], f32r,
                               tag=("accr" if r == 0 else "m1b"),
                               name=f"m1_{p}_{r}"))
        for og in range(4):
            pss = {}
            for o2 in range(2):
                for r in range(n_chunk):
                    pss[o2, r] = Pps.tile([128, CHUNK], f32, tag="ps",
                                          name=f"pg{p}_{og}_{o2}_{r}")
            for k in range(KH):
                for o2 in range(2):
                    o = og * 2 + o2
                    for r in range(n_chunk):
                        nc.tensor.matmul(
                            pss[o2, r][:],
                            g1w[:, k * 1024 + o * 128:k * 1024 + o * 128 + 128],
                            ht[:, k * pass_rows + r * CHUNK:k * pass_rows + r * CHUNK + CHUNK],
                            start=(k == 0), stop=(k == KH - 1))
            for o2 in range(2):
                o = og * 2 + o2
                for r in range(n_chunk):
                    nc.vector.tensor_scalar(
                        m1s[r][:, o * CHUNK:(o + 1) * CHUNK],
                        pss[o2, r][:], g1b[:, o:o + 1], 0.0,
                        op0=Op.add, op1=Op.max)
        # Softmax with deferred normalization: wd/wr hold UNNORMALIZED exp;
        # 1/sum lands in recd/recr and is applied to the head outputs in
        # phase C (a per-column scale commutes through the head matmul).
        # This keeps the slow 1-partition reciprocal off the PE critical path.
        rec_all = P1.tile([1, 2 * pass_rows], f32r, tag="rec", name=f"rec{p}")
        recd = rec_all[:, 0:pass_rows]
        recr = rec_all[:, pass_rows:2 * pass_rows]
        gcfg = [(r, gi) for r in range(n_chunk) for gi in range(2)]
        gmeta = {0: ((0, 8), g2bd, wd, recd, io["wdynT"]),
                 1: ((8, 16), g2br, wr, recr, io["wrewT"])}
        pgs = {}
        for r, gi in gcfg:
            gslice = gmeta[gi][0]
            pg = Pps.tile([8, CHUNK], f32, tag="ps", name=f"pgg{p}_{r}_{gi}")
            for k in range(KH):
                nc.tensor.matmul(
                    pg[:], g2w[:, k * 16 + gslice[0]:k * 16 + gslice[1]],
                    m1s[r][:, k * CHUNK:(k + 1) * CHUNK],
                    start=(k == 0), stop=(k == KH - 1))
            pgs[r, gi] = pg
            nc.scalar.activation(gmeta[gi][2][:, r * CHUNK:(r + 1) * CHUNK],
                                 pg[:], Act.Exp, bias=gmeta[gi][1][:, 0:1])
        scps = {}
        for r, gi in gcfg:
            wt = gmeta[gi][2]
            psums = Pps.tile([1, CHUNK], f32, tag="ps", name=f"psum{p}_{r}_{gi}")
            nc.tensor.matmul(psums[:], ones8[:],
                             wt[:, r * CHUNK:(r + 1) * CHUNK],
                             start=True, stop=True)
            # fast PSUM->SBUF copy on ACT releases the bank; the slow
            # 1-partition reciprocal then runs off the critical path
            scp = Ptmp.tile([1, CHUNK], f32, tag="tmp", name=f"scp{p}_{r}_{gi}")
            nc.scalar.activation(scp[:], psums[:], Act.Copy)
            scps[r, gi] = scp

        # ================= phase B: experts =================
        accd = P1.tile([128, KD * pass_rows], f32r, tag="ztacc", name=f"accd{p}")
        accr = P1.tile([128, KD * pass_rows], f32r, tag="accr", name=f"accr{p}")
        # init acc with gate-weighted eo bias: acc[d, n] = sum_e eo_b[e, d] w[e, n]
        for r in range(n_chunk):
            for d in range(KD):
                pi = Pps.tile([128, CHUNK], f32, tag="ps", name=f"pid{p}_{r}_{d}")
                nc.tensor.matmul(pi[:], ebt[:, d * 128:(d + 1) * 128],
                                 wd[:, r * CHUNK:(r + 1) * CHUNK],
                                 start=True, stop=True)
                nc.scalar.activation(
                    accd[:, d * pass_rows + r * CHUNK:d * pass_rows + r * CHUNK + CHUNK],
                    pi[:], Act.Copy)
                pi2 = Pps.tile([128, CHUNK], f32, tag="ps", name=f"pir{p}_{r}_{d}")
                nc.tensor.matmul(pi2[:], ebt[:, d * 128:(d + 1) * 128],
                                 wr[:, r * CHUNK:(r + 1) * CHUNK],
                                 start=True, stop=True)
                nc.vector.tensor_copy(
                    accr[:, d * pass_rows + r * CHUNK:d * pass_rows + r * CHUNK + CHUNK],
                    pi2[:])

        # reciprocals after the init block, as cheap approx ops (sums are
        # well-conditioned; 18 bits is far beyond the f32r noise floor)
        for r, gi in gcfg:
            rec = gmeta[gi][3]
            rf = Pbc.tile([1, CHUNK], f32, tag="bc", name=f"rf{p}_{r}_{gi}")
            nc.vector.reciprocal_approx_fast(rf[:], scps[r, gi][:])
            nc.vector.tensor_copy(rec[:, r * CHUNK:(r + 1) * CHUNK], rf[:])

        for e in range(E):
            # broadcast gate weights across partitions via one-hot matmul
            bc = {}
            for gi, wt in ((0, wd), (1, wr)):
                for r in range(n_chunk):
                    pb = Pps.tile([128, CHUNK], f32, tag="ps",
                                  name=f"pb{p}_{e}_{gi}_{r}")
                    nc.tensor.matmul(pb[:], sel[:, e * 128:(e + 1) * 128],
                                     wt[:, r * CHUNK:(r + 1) * CHUNK],
                                     start=True, stop=True)
                    t = Pbc.tile([128, CHUNK], f32, tag="bc",
                                 name=f"bc{p}_{e}_{gi}_{r}")
                    nc.vector.tensor_copy(t[:], pb[:])
                    bc[gi, r] = t
            e1b = Pws.tile([128, 8], f32, tag="eb1", name=f"e1b{p}_{e}")
            nc.gpsimd.dma_start(e1b[:], io["e1_b"][e].rearrange("(o p) -> p o", p=128))
            e2b = Pws.tile([128, 8], f32, tag="eb2", name=f"e2b{p}_{e}")
            nc.gpsimd.dma_start(e2b[:], io["e2_b"][e].rearrange("(o p) -> p o", p=128))

            # x1 = relu(e1_W[e].T @ h + e1_b[e])
            x1 = P1.tile([128, KH * pass_rows], f32r, tag="A1", name=f"x1_{p}_{e}")
            for og in range(4):
                wsth = []
                for hh in range(2):
                    t = Pws.tile([128, 4 * 256], f32r, tag="ws",
                                 name=f"w1_{p}_{e}_{og}_{hh}")
                    nc.sync.dma_start(
                        t[:], e1W[e, hh * 512:(hh + 1) * 512,
                                  og * 256:(og + 1) * 256]
                        .rearrange("(k p) m -> p k m", p=128))
                    wsth.append(t)
                pss = {}
                for o2 in range(2):
                    for r in range(n_chunk):
                        pss[o2, r] = Pps.tile([128, CHUNK], f32, tag="ps",
                                              name=f"px1{p}_{e}_{og}_{o2}_{r}")
                for k in range(KH):
                    for o2 in range(2):
                        for r in range(n_chunk):
                            nc.tensor.matmul(
                                pss[o2, r][:],
                                wsth[k // 4][:, (k % 4) * 256 + o2 * 128:(k % 4) * 256 + o2 * 128 + 128],
                                ht[:, k * pass_rows + r * CHUNK:k * pass_rows + r * CHUNK + CHUNK],
                                start=(k == 0), stop=(k == KH - 1))
                for o2 in range(2):
                    o = og * 2 + o2
                    for r in range(n_chunk):
                        nc.scalar.activation(
                            x1[:, o * pass_rows + r * CHUNK:o * pass_rows + r * CHUNK + CHUNK],
                            pss[o2, r][:], Act.Relu, bias=e1b[:, o:o + 1])

            if e == 0:
                # normalized gate-weight outputs; emitted an expert-stage in
                # so the reciprocals (slow, DVE) are long done
                for r, gi in gcfg:
                    wt, rec, wout = gmeta[gi][2], gmeta[gi][3], gmeta[gi][4]
                    pb8 = Pps.tile([8, CHUNK], f32, tag="ps",
                                   name=f"pb8{p}_{r}_{gi}")
                    nc.tensor.matmul(pb8[:], ones18[:],
                                     rec[:, r * CHUNK:(r + 1) * CHUNK],
                                     start=True, stop=True)
                    wn = Pout.tile([8, CHUNK], f32r, tag="out",
                                   name=f"wn{p}_{r}_{gi}")
                    nc.vector.tensor_tensor(wn[:],
                                            wt[:, r * CHUNK:(r + 1) * CHUNK],
                                            pb8[:], op=Op.mult)
                    nc.scalar.dma_start(
                        wout[:, ps + r * CHUNK:ps + (r + 1) * CHUNK], wn[:])

            # x2 = relu(e2_W[e].T @ x1 + e2_b[e])
            x2 = P1.tile([128, KH * pass_rows], f32r, tag="A2", name=f"x2_{p}_{e}")
            for og in range(4):
                wsth = []
                for hh in range(2):
                    t = Pws.tile([128, 4 * 256], f32r, tag="ws",
                                 name=f"w2_{p}_{e}_{og}_{hh}")
                    nc.sync.dma_start(
                        t[:], e2W[e, hh * 512:(hh + 1) * 512,
                                  og * 256:(og + 1) * 256]
                        .rearrange("(k p) m -> p k m", p=128))
                    wsth.append(t)
                pss = {}
                for o2 in range(2):
                    for r in range(n_chunk):
                        pss[o2, r] = Pps.tile([128, CHUNK], f32, tag="ps",
                                              name=f"px2{p}_{e}_{og}_{o2}_{r}")
                for k in range(KH):
                    for o2 in range(2):
                        for r in range(n_chunk):
                            nc.tensor.matmul(
                                pss[o2, r][:],
                                wsth[k // 4][:, (k % 4) * 256 + o2 * 128:(k % 4) * 256 + o2 * 128 + 128],
                                x1[:, k * pass_rows + r * CHUNK:k * pass_rows + r * CHUNK + CHUNK],
                                start=(k == 0), stop=(k == KH - 1))
                for o2 in range(2):
                    o = og * 2 + o2
                    for r in range(n_chunk):
                        nc.vector.tensor_scalar(
                            x2[:, o * pass_rows + r * CHUNK:o * pass_rows + r * CHUNK + CHUNK],
                            pss[o2, r][:], e2b[:, o:o + 1], 0.0,
                            op0=Op.add, op1=Op.max)

            # feats = eo_W[e].T @ x2; acc += w[e] * feats (both gates)
            for dg in range(2):
                wsth = []
                for hh in range(2):
                    t = Pws.tile([128, 4 * 256], f32r, tag="ws",
                                 name=f"wo_{p}_{e}_{dg}_{hh}")
                    nc.sync.dma_start(
                        t[:], eoW[e, hh * 512:(hh + 1) * 512,
                                  dg * 256:(dg + 1) * 256]
                        .rearrange("(k p) m -> p k m", p=128))
                    wsth.append(t)
                pfs = {}
                for d2 in range(2):
                    for r in range(n_chunk):
                        pfs[d2, r] = Pps.tile([128, CHUNK], f32, tag="ps",
                                              name=f"pf{p}_{e}_{dg}_{d2}_{r}")
                for k in range(KH):
                    for d2 in range(2):
                        for r in range(n_chunk):
                            nc.tensor.matmul(
                                pfs[d2, r][:],
                                wsth[k // 4][:, (k % 4) * 256 + d2 * 128:(k % 4) * 256 + d2 * 128 + 128],
                                x2[:, k * pass_rows + r * CHUNK:k * pass_rows + r * CHUNK + CHUNK],
                                start=(k == 0), stop=(k == KH - 1))
                for d2 in range(2):
                    d = dg * 2 + d2
                    for r in range(n_chunk):
                        c0 = d * pass_rows + r * CHUNK
                        tmp = Ptmp.tile([128, CHUNK], f32, tag="tmp",
                                        name=f"td{p}_{e}_{dg}_{d2}_{r}")
                        nc.vector.tensor_tensor(tmp[:], pfs[d2, r][:],
                                                bc[0, r][:], op=Op.mult)
                        # acc += tmp on GpSimd: keeps DVE free for the PSUM
                        # reads, so PSUM banks recycle fast at eo boundaries
                        nc.gpsimd.tensor_add(accd[:, c0:c0 + CHUNK],
                                             accd[:, c0:c0 + CHUNK], tmp[:])
                        tmp2 = Ptmp.tile([128, CHUNK], f32, tag="tmp",
                                         name=f"tr{p}_{e}_{dg}_{d2}_{r}")
                        nc.vector.tensor_tensor(tmp2[:], pfs[d2, r][:],
                                                bc[1, r][:], op=Op.mult)
                        nc.gpsimd.tensor_add(accr[:, c0:c0 + CHUNK],
                                             accr[:, c0:c0 + CHUNK], tmp2[:])

        # ================= phase C: fused heads =================
        hw0h = []
        for hh in range(2):
            t = Pws.tile([128, 2 * 512], f32r, tag="ws", name=f"hw0_{p}_{hh}")
            nc.sync.dma_start(
                t[:], io["head_W0"][hh * 256:(hh + 1) * 256, :]
                .rearrange("(k p) m -> p k m", p=128))
            hw0h.append(t)
        hw1 = Pws.tile([128, 4 * RDIM], f32r, tag="ws", name=f"hw1_{p}")
        nc.sync.dma_start(hw1[:], io["head_W1"].rearrange("(k p) m -> p k m", p=128))
        for r in range(n_chunk):
            # broadcast 1/sum to 128 partitions for the deferred normalization
            rb = {}
            for gi, rec in ((0, recd), (1, recr)):
                pbc = Pps.tile([128, CHUNK], f32, tag="ps", name=f"pbc{p}_{r}_{gi}")
                nc.tensor.matmul(pbc[:], ones1x128[:],
                                 rec[:, r * CHUNK:(r + 1) * CHUNK],
                                 start=True, stop=True)
                t = Pbc.tile([128, CHUNK], f32, tag="bc", name=f"rb{p}_{r}_{gi}")
                nc.vector.tensor_copy(t[:], pbc[:])
                rb[gi] = t
            for d in range(4):
                ph = Pps.tile([128, CHUNK], f32, tag="ps", name=f"ph{p}_{r}_{d}")
                for k in range(KD):
                    nc.tensor.matmul(
                        ph[:], hw0h[k // 2][:, (k % 2) * 512 + d * 128:(k % 2) * 512 + d * 128 + 128],
                        accd[:, k * pass_rows + r * CHUNK:k * pass_rows + r * CHUNK + CHUNK],
                        start=(k == 0), stop=(k == KD - 1))
                t1 = Ptmp.tile([128, CHUNK], f32, tag="tmp", name=f"th{p}_{r}_{d}")
                nc.vector.tensor_tensor(t1[:], ph[:], rb[0][:], op=Op.mult)
                ot = Pout.tile([128, CHUNK], f32, tag="out", name=f"ot{p}_{r}_{d}")
                nc.vector.tensor_scalar_add(ot[:], t1[:], hb0[:, d:d + 1])
                eng = nc.scalar if d % 2 == 0 else nc.sync
                eng.dma_start(
                    io["nzT"][d * 128:(d + 1) * 128, ps + r * CHUNK:ps + r * CHUNK + CHUNK],
                    ot[:])
            pr = Pps.tile([RDIM, CHUNK], f32, tag="ps", name=f"prew{p}_{r}")
            for k in range(KD):
                nc.tensor.matmul(
                    pr[:], hw1[:, k * RDIM:(k + 1) * RDIM],
                    accr[:, k * pass_rows + r * CHUNK:k * pass_rows + r * CHUNK + CHUNK],
                    start=(k == 0), stop=(k == KD - 1))
            t2 = Ptmp.tile([RDIM, CHUNK], f32, tag="tmp", name=f"thr{p}_{r}")
            nc.vector.tensor_tensor(t2[:], pr[:], rb[1][0:RDIM, :], op=Op.mult)
            orw = Pout.tile([RDIM, CHUNK], f32, tag="out", name=f"orw{p}_{r}")
            nc.vector.tensor_scalar_add(orw[:], t2[:], hb1[:, 0:1])
            nc.sync.dma_start(
                io["rewT"][:, ps + r * CHUNK:ps + r * CHUNK + CHUNK], orw[:])


def _build(rows, n_cores):
    import concourse.bacc as bacc
    import concourse.tile as tile
    import concourse.mybir as mybir
    f32 = mybir.dt.float32
    f32r = mybir.dt.float32r

    nc = bacc.Bacc("TRN2", target_bir_lowering=False, debug=False,
                   num_devices=n_cores)
    io = {}

    def inp(name, shape, dt):
        io[name] = nc.dram_tensor(name, shape, dt, kind="ExternalInput").ap()

    def outp(name, shape, dt):
        io[name] = nc.dram_tensor(name, shape, dt, kind="ExternalOutput").ap()

    inp("zT", [LATENT, rows], f32r)
    inp("aT", [ACTION, rows], f32r)
    inp("pre_W", [LATENT + ACTION, HIDDEN], f32r)
    inp("pre_b", [8, 128], f32)
    inp("g1_W", [HIDDEN, HIDDEN], f32r)
    inp("g1_b", [8, 128], f32)
    inp("g2_W", [HIDDEN, 2 * E], f32r)
    inp("g2_b", [2 * E, 1], f32)
    inp("e1_W", [E, HIDDEN, HIDDEN], f32r)
    inp("e1_b", [E, HIDDEN], f32)
    inp("e2_W", [E, HIDDEN, HIDDEN], f32r)
    inp("e2_b", [E, HIDDEN], f32)
    inp("eo_W", [E, HIDDEN, LATENT], f32r)
    inp("eo_b", [E, LATENT], f32r)
    inp("head_W0", [LATENT, LATENT], f32r)
    inp("head_W1", [LATENT, RDIM], f32r)
    inp("head_b0", [4, 128], f32)
    inp("head_b1", [RDIM, 1], f32)
    inp("c_ones8", [8, 1], f32r)
    inp("c_ones18", [1, 8], f32r)
    inp("c_ones1x128", [1, 128], f32r)
    inp("c_sel", [8, E * 128], f32r)

    outp("nzT", [LATENT, rows], f32)
    outp("rewT", [RDIM, rows], f32)
    outp("wdynT", [E, rows], f32r)
    outp("wrewT", [E, rows], f32r)

    from contextlib import ExitStack
    with tile.TileContext(nc) as tc:
        with ExitStack() as ctx:
            _emit(nc, tc, io, rows, ctx)
    nc.compile()
    return nc


def _host_inputs(z, a, pre_W, pre_b, g1_W, g1_b, g2_W, g2_b,
                 e1_W, e1_b, e2_W, e2_b, eo_W, eo_b, head_W, head_b,
                 rows, n_cores):
    """Build per-core in_maps (host-side transposes/shaping)."""
    zT = np.ascontiguousarray(z.T)
    aT = np.ascontiguousarray(a.T)
    sel_np = np.zeros((8, E * 128), np.float32)
    for e in range(E):
        sel_np[e, e * 128:(e + 1) * 128] = 1.0
    shared = {
        "pre_W": np.ascontiguousarray(pre_W),
        "pre_b": np.ascontiguousarray(pre_b.reshape(8, 128)),
        "g1_W": np.ascontiguousarray(g1_W),
        "g1_b": np.ascontiguousarray(g1_b.reshape(8, 128)),
        "g2_W": np.ascontiguousarray(g2_W),
        "g2_b": np.ascontiguousarray(g2_b.reshape(2 * E, 1)),
        "e1_W": np.ascontiguousarray(e1_W),
        "e1_b": np.ascontiguousarray(e1_b),
        "e2_W": np.ascontiguousarray(e2_W),
        "e2_b": np.ascontiguousarray(e2_b),
        "eo_W": np.ascontiguousarray(eo_W),
        "eo_b": np.ascontiguousarray(eo_b),
        "head_W0": np.ascontiguousarray(head_W[0, :, :LATENT]),
        "head_W1": np.ascontiguousarray(head_W[1, :, :RDIM]),
        "head_b0": np.ascontiguousarray(head_b[0, :LATENT].reshape(4, 128)),
        "head_b1": np.ascontiguousarray(head_b[1, :RDIM].reshape(RDIM, 1)),
        "c_ones8": np.ones((8, 1), np.float32),
        "c_ones18": np.ones((1, 8), np.float32),
        "c_ones1x128": np.ones((1, 128), np.float32),
        "c_sel": sel_np,
    }
    in_maps = []
    for c in range(n_cores):
        m = dict(shared)
        m["zT"] = np.ascontiguousarray(zT[:, c * rows:(c + 1) * rows])
        m["aT"] = np.ascontiguousarray(aT[:, c * rows:(c + 1) * rows])
        in_maps.append(m)
    return in_maps


def kernel(z, a, pre_W, pre_b, g1_W, g1_b, g2_W, g2_b,
           e1_W, e1_b, e2_W, e2_b, eo_W, eo_b, head_W, head_b):
    from concourse.bass_utils import run_bass_kernel_spmd

    args = [np.asarray(x, dtype=np.float32) for x in
            (z, a, pre_W, pre_b, g1_W, g1_b, g2_W, g2_b,
             e1_W, e1_b, e2_W, e2_b, eo_W, eo_b, head_W, head_b)]
    key = ("full", ROWS, N_CORES)
    if key not in _cache:
        _cache[key] = _build(ROWS, N_CORES)
    nc = _cache[key]
    in_maps = _host_inputs(*args, rows=ROWS, n_cores=N_CORES)
    res = run_bass_kernel_spmd(nc, in_maps, core_ids=list(range(N_CORES)))

    next_z = np.empty((N, LATENT), np.float32)
    reward = np.empty((N, RDIM), np.float32)
    w_dyn = np.empty((N, E), np.float32)
    w_rew = np.empty((N, E), np.float32)
    for c in range(N_CORES):
        r = res.results[c]
        sl = slice(c * ROWS, (c + 1) * ROWS)
        next_z[sl] = r["nzT"].T
        reward[sl] = r["rewT"].T
        w_dyn[sl] = r["wdynT"].T
        w_rew[sl] = r["wrewT"].T
    next_z += args[0]          # residual added on host
    return next_z, reward, w_dyn, w_rew
